# revision 21
# baseline (speedup 1.0000x reference)
"""MultiHeadAttention TRN2 Bass kernel (nn_MultiHeadAttention, B=4 S=2048 E=1024 H=16).

Sharding over 8 NeuronCores: core c -> (batch b = c//2, head-half hh = c%2).
Each core computes, for its batch and its 8 heads: the Q/K/V projections,
attention, and a partial out-projection over its 512 "dk" dims with bo/2
added; the host sums the two partials per batch (Megatron tensor-parallel
with the all-reduce replaced by a host-side pair sum).  All 8 cores run one
identical SPMD program on different data.

On-device layout (contraction-major / "T" = k-major):
  QT, KT   [dims 512, S] fp16, 4 tiles [128, S]; pair p = heads (2p, 2p+1):
           head A on partitions 0-63, head B on 64-127.
  V'       [128, m2-tile 16, head 8, 65] fp16; col 64 of each head block is
           1.0, so the PV matmul emits softmax denominators for free.
  scoresT  [m2 128, m1 512] in PSUM (row-tiled pair matmuls, K=64,
           tile_position (0,0)/(64,0)); exp on ACT, scale=1/8 fused, no
           max-subtraction (scores ~ N(0,1), max < 7 over 268M samples).
  PV       out_h^T [65, m1] = V'_h^T @ P_h^T accumulated over 16 m2 tiles
           (P in fp16 from the exp).
  divide   s-row roundtripped through scratch DRAM with a partition-
           broadcast read AP + DVE reciprocal_approx_fast + multiply.
  out-proj fp16: psum[m 128, n 512] = sum_dk aout[dk, m]^T wo[dk, n], + bo/2
           via a K=1 ones-outer-product matmul.

All matmuls run in fp16 (e5m10 — every operand is O(1)-O(400), so range is
safe and the mantissa beats bf16 by 8x); accumulation is fp32 in PSUM.
Measured end-to-end rel err vs the fp32 reference: ~7e-4.  Q-proj and the
out-projection are interleaved into the attention pair loop, one psum-group
between m2-groups, so PE and ACT (the two near-critical engines, ~420us and
~280us busy) stay fed across pair and chunk boundaries.  HW exec time
measured at ~459us/core across 8 cores.
"""

import numpy as np

import concourse.bass as bass
import concourse.mybir as mybir
import concourse.tile as tile
from concourse import bacc

F32 = mybir.dt.float32
F32R = mybir.dt.float32r
F16 = mybir.dt.float16
I16 = mybir.dt.int16
AF = mybir.ActivationFunctionType

# Schraudolph fast-exp constants: fp16 bitpattern of exp(s * 0.125) is
# approximately s * (0.125 * log2(e) * 1024) + (15 * 1024 + rounding/centre
# correction).  Max relative error ~3.5% on the tiles routed to DVE.
SCHRAUD_A = 0.125 * 1.4426950408889634 * 1024.0
SCHRAUD_B = 15360.0 - 34.8 + 0.5

B, S, E, H, D = 4, 2048, 1024, 16, 64
HS = 512            # dims per core (8 heads)
PAIRS = 4           # head pairs per core
MC = 512            # m1 chunk
NMC = S // MC       # 4
NKT = E // 128      # 8 contraction chunks for projections
NMT = S // 128      # 16 m2 tiles
PROJ_F16 = True    # projections in fp16 too (vs fp32r)
PDT = F16 if PROJ_F16 else F32R


def round_fp32r(a):
    """Round-to-nearest-even to e8m11 (fp32 with low 12 mantissa bits zero)."""
    bits = np.ascontiguousarray(a, np.float32).view(np.uint32)
    lsb = (bits >> 12) & 1
    out = (bits + 0x7FF + lsb) & 0xFFFFF000
    return out.view(np.float32)


def _dedup_ldweights(nc):
    """Remove an InstLdweights when the previous PE weight load in the same
    block loaded the exact same AP (only Matmults, which don't disturb the
    array, may sit between).  Engines execute block order, so the PE array
    still holds those weights.  Sync info from the removed load is moved to
    the next instruction."""
    import concourse.mybir as mybir

    removed = 0
    for fn in nc.m.functions:
        for blk in fn.blocks:
            insts = list(blk.instructions)
            keep = []
            last_w = None
            pending_sync = None
            for i in insts:
                if i.opcode == "Ldweights":
                    w = str(i.ins[0])
                    if w == last_w:
                        si = i.sync_info
                        if si is not None and (si.on_wait or si.on_update):
                            pending_sync = si
                        removed += 1
                        continue
                    last_w = w
                elif i.opcode == "Matmult":
                    pass  # uses loaded weights, doesn't clobber them
                elif i.opcode in ("TensorCopy", "TensorScalarPtr",
                                  "TensorTensor", "Activation", "DMACopy",
                                  "Memset", "ISA", "EventSemaphore"):
                    pass  # other engines don't touch the PE array
                else:
                    last_w = None  # control flow etc.: be conservative
                if pending_sync is not None:
                    si = i.sync_info
                    if si is None:
                        i.sync_info = pending_sync
                    else:
                        for w_ in pending_sync.on_wait:
                            si.on_wait.append(w_)
                        for u_ in pending_sync.on_update:
                            si.on_update.append(u_)
                    pending_sync = None
                keep.append(i)
            if removed and len(keep) != len(insts):
                blk.set_instructions(keep) if hasattr(blk, "set_instructions") \
                    else setattr(blk, "instructions", keep)
    return removed


def build_nc():
    nc = bacc.Bacc()

    xq_d = nc.dram_tensor("xq_t", [E, S], PDT, kind="ExternalInput")
    xk_d = nc.dram_tensor("xk_t", [E, S], PDT, kind="ExternalInput")
    xv_d = nc.dram_tensor("xv_t", [E, S], PDT, kind="ExternalInput")
    wq_d = nc.dram_tensor("wq_t", [E, HS], PDT, kind="ExternalInput")
    wk_d = nc.dram_tensor("wk_t", [E, HS], PDT, kind="ExternalInput")
    wv_d = nc.dram_tensor("wv_t", [E, HS], PDT, kind="ExternalInput")
    wo_d = nc.dram_tensor("wo_t", [HS, E], F16, kind="ExternalInput")
    bq_d = nc.dram_tensor("bq", [HS], F32, kind="ExternalInput")
    # bk is dropped entirely: adding bk to k shifts every score row by a
    # constant (q_i . bk, independent of key j), which cancels in softmax.
    # bv is folded host-side into bo_eff (P @ (V + 1 bv^T))/den = PV/den + bv,
    # so bv @ Wo^T joins the output bias.  bo_row carries bo_eff in f32.
    bo_d = nc.dram_tensor("bo_row", [1, E], F32, kind="ExternalInput")

    out_d = nc.dram_tensor("out_partial", [S, E], F32, kind="ExternalOutput")
    scratch_d = nc.dram_tensor("scratch_v4", [S // (2 * MC), PAIRS, 2, 2 * MC],
                               F32)

    def bcast_ap(row_ap, n):
        return bass.AP(tensor=row_ap.tensor, offset=row_ap.offset,
                       ap=[[0, n]] + list(row_ap.ap[1:]))

    with tile.TileContext(nc) as tc:
        with (
            tc.tile_pool(name="const", bufs=1) as const,
            tc.tile_pool(name="qkv", bufs=1) as qkv,
            tc.tile_pool(name="aout", bufs=1) as aoutp,
        ):
            bq_sb = const.tile([128, PAIRS], F32)
            nc.sync.dma_start(bq_sb[:], bq_d.rearrange("(t p) -> p t", p=128))
            bo_bc = const.tile([128, E], F32)
            nc.sync.dma_start(bo_bc[:], bcast_ap(bo_d[:], 128))

            qt_all = qkv.tile([128, PAIRS, S], F16, tag="qt")
            kt_all = qkv.tile([128, PAIRS, S], F16, tag="kt")
            v_all = qkv.tile([128, NMT, 8, 65], F16, tag="v")
            nc.vector.memset(v_all[:, :, :, 64], 1.0)

            aout = [aoutp.tile([128, S], F16, name=f"aout{p}", tag=f"ao{p}")
                    for p in range(PAIRS)]

            with (
                tc.tile_pool(name="w", bufs=2) as wpool,
                tc.tile_pool(name="x", bufs=2) as xpool,
            ):
                # ======== K and V projections (own PSUM scope) ========
                # K-proj uses compound matmuls: rhs covers a 2-chunk (1024
                # token) span so each weight load serves 2 bank-matmuls.
                with tc.tile_pool(name="pp", bufs=2,
                                  space=bass.MemorySpace.PSUM) as pp:
                    wk_sb = wpool.tile([128, NKT, HS], PDT, tag="w")
                    nc.sync.dma_start(
                        wk_sb[:], wk_d.rearrange("(kc p) n -> p kc n", p=128))

                    for mch in range(NMC // 2):
                        msl = slice(mch * 2 * MC, (mch + 1) * 2 * MC)
                        x_t = xpool.tile([128, NKT, 2 * MC], PDT, tag="x")
                        nc.sync.dma_start(
                            x_t[:],
                            xk_d.rearrange("(kc p) m -> p kc m", p=128)[
                                :, :, msl
                            ],
                        )
                        for nt in range(PAIRS):
                            ps = pp.tile([128, 2, MC], F32, tag="ppk")
                            for kc in range(NKT):
                                for h in range(2):
                                    nc.tensor.matmul(
                                        ps[:, h, :],
                                        wk_sb[:, kc, nt * 128:(nt + 1) * 128],
                                        x_t[:, kc, h * MC:(h + 1) * MC],
                                        start=(kc == 0),
                                        stop=(kc == NKT - 1),
                                    )
                            nc.vector.tensor_copy(
                                kt_all[:, nt, msl], ps[:])

                    wv_sb = wpool.tile([128, NKT, HS], PDT, tag="w")
                    nc.sync.dma_start(
                        wv_sb[:], wv_d.rearrange("(kc p) n -> p kc n", p=128))
                    for mch in range(NMC // 2):
                        msl = slice(mch * 2 * MC, (mch + 1) * 2 * MC)
                        x_t = xpool.tile([128, NKT, 2 * MC], PDT, tag="x")
                        nc.sync.dma_start(
                            x_t[:],
                            xv_d.rearrange("(kc p) m -> p kc m", p=128)[
                                :, :, msl
                            ],
                        )
                        for mt_l in range(2 * MC // 128):
                            mt = mch * (2 * MC // 128) + mt_l
                            ps = pp.tile([128, HS], F32, tag="ppv", bufs=4)
                            for kc in range(NKT):
                                nc.tensor.matmul(
                                    ps[:],
                                    x_t[:, kc, mt_l * 128:(mt_l + 1) * 128],
                                    wv_sb[:, kc, :],
                                    start=(kc == 0),
                                    stop=(kc == NKT - 1),
                                )
                            nc.vector.tensor_copy(
                                v_all[:, mt, :, 0:64],
                                ps[:].rearrange("p (h c) -> p h c", c=64),
                            )

                # ======== Q-proj interleaved with attention + out-proj ========
                # Attention runs per (pair, mch) where mch is a 1024-token
                # query chunk: the kt / V stationaries are each loaded once
                # and reused for two 512-col matmuls (the second LDWEIGHTS is
                # deduped post-scheduling).  PSUM: tag "sc" 2x[128,2,512]
                # (scores + interleaved q-proj/out-proj psum) + tag "pv"
                # 2x[65,2,512] (two heads' PV accumulators) = 8 banks.
                wq_sb = wpool.tile([128, NKT, HS], PDT, tag="w")
                nc.sync.dma_start(
                    wq_sb[:], wq_d.rearrange("(kc p) n -> p kc n", p=128))

                MH = 2 * MC          # 1024-token query chunk
                NMH = S // MH        # 2 chunks

                with (
                    tc.tile_pool(name="wo", bufs=1) as wop,
                    tc.tile_pool(name="pt", bufs=3) as ptp,
                    tc.tile_pool(name="msc", bufs=2) as msc,
                    tc.tile_pool(name="ost", bufs=4) as ostp,
                    tc.tile_pool(name="sc", bufs=2,
                                 space=bass.MemorySpace.PSUM) as scp,
                    tc.tile_pool(name="pv", bufs=2,
                                 space=bass.MemorySpace.PSUM) as pvp,
                ):
                    wo_sb = wop.tile([128, PAIRS, E], F16, tag="wo")
                    nc.sync.dma_start(
                        wo_sb[:], wo_d.rearrange("(dk p) n -> p dk n", p=128))

                    qx = {}

                    def qproj_x(mh):
                        x_t = xpool.tile([128, NKT, MH], PDT, tag="x",
                                         name=f"xq{mh}")
                        nc.sync.dma_start(
                            x_t[:],
                            xq_d.rearrange("(kc p) m -> p kc m", p=128)[
                                :, :, mh * MH:(mh + 1) * MH
                            ],
                        )
                        qx[mh] = x_t

                    def qproj_group(mh, nt):
                        x_t = qx[mh]
                        ps = scp.tile([128, 2, MC], F32, tag="sc", name="qps")
                        for kc in range(NKT):
                            for h in range(2):
                                nc.tensor.matmul(
                                    ps[:, h, :],
                                    wq_sb[:, kc, nt * 128:(nt + 1) * 128],
                                    x_t[:, kc, h * MC:(h + 1) * MC],
                                    start=(kc == 0),
                                    stop=(kc == NKT - 1),
                                )
                        nc.vector.tensor_scalar_add(
                            qt_all[:, nt, mh * MH:(mh + 1) * MH],
                            ps[:],
                            bq_sb[:, nt:nt + 1],
                        )

                    def attention(mh, pair, extra=()):
                        extra = list(extra)
                        m1 = slice(mh * MH, (mh + 1) * MH)
                        pvt = [pvp.tile([65, 2, MC], F32, name=f"pv{h}",
                                        tag="pv") for h in range(2)]
                        pt = [None, None]
                        for t in range(NMT):
                            m2 = slice(t * 128, (t + 1) * 128)
                            sct = [scp.tile([128, 2, MC], F32, tag="sc",
                                            name=f"sc{bi}") for bi in range(2)]
                            # order A0 B0 B1 A1: cross-band neighbors
                            # co-execute on the PE row bands; B's pair and
                            # A's... B1 reuses B0's kt (LDWEIGHTS deduped)
                            for bi, h in ((0, 0), (1, 0), (1, 1), (0, 1)):
                                band = slice(64 * bi, 64 * (bi + 1))
                                nc.tensor.matmul(
                                    sct[bi][:, h, :],
                                    kt_all[band, pair, m2],
                                    qt_all[band, pair,
                                           mh * MH + h * MC:
                                           mh * MH + (h + 1) * MC],
                                    start=True, stop=True,
                                    tile_position=(64 * bi, 0),
                                )
                            for bi in range(2):
                                if bi == 1 and t % 2 == 1:
                                    # Schraudolph exp on DVE: fp16 bits of
                                    # exp(s/8) ~= s*(log2e/8*1024) + bias.
                                    # Keeps ACT from being the per-iteration
                                    # bottleneck (PE must never idle or it
                                    # drops out of its top p-state).
                                    p = ptp.tile([128, 2, MC], I16, tag="pt",
                                                 name=f"pt{bi}")
                                    nc.vector.tensor_scalar(
                                        p[:], sct[bi][:],
                                        SCHRAUD_A, SCHRAUD_B,
                                        op0=mybir.AluOpType.mult,
                                        op1=mybir.AluOpType.add,
                                    )
                                    pt[bi] = p.bitcast(F16)
                                else:
                                    p = ptp.tile([128, 2, MC], F16, tag="pt",
                                                 name=f"pt{bi}")
                                    nc.scalar.activation(
                                        p[:], sct[bi][:], AF.Exp, scale=0.125)
                                    pt[bi] = p
                            for bi in range(2):
                                for h in range(2):
                                    nc.tensor.matmul(
                                        pvt[bi][:, h, :],
                                        v_all[:, t, 2 * pair + bi, :],
                                        pt[bi][:, h, :],
                                        start=(t == 0), stop=(t == NMT - 1),
                                    )
                            if t in (4, 9, 14) and extra:
                                extra.pop(0)()
                        while extra:
                            extra.pop(0)()

                        # normalize: out_h = pv[0:64] / pv[64] (row 64 holds
                        # the softmax denominators via the ones-column of V').
                        for h in range(2):
                            pvs = msc.tile([65, 2, MC], F32, name=f"pvs{h}",
                                           tag="pvs")
                            nc.vector.tensor_copy(pvs[:], pvt[h][:])
                            srow_dram = scratch_d[mh:mh + 1, pair, h, :]
                            nc.sync.dma_start(srow_dram,
                                              pvs[64:65, :, :])
                            bc = msc.tile([64, MH], F32, tag="bc")
                            nc.sync.dma_start(bc[:], bcast_ap(srow_dram, 64))
                            inv = msc.tile([64, MH], F32, tag="inv")
                            nc.vector.reciprocal_approx_fast(inv[:], bc[:])
                            if h == 0:
                                nc.vector.tensor_mul(
                                    aout[pair][0:64, m1], pvs[0:64, :, :],
                                    inv[:],
                                )
                            else:
                                tmpb = msc.tile([64, MH], F16, tag="tmpb")
                                nc.vector.tensor_mul(tmpb[:], pvs[0:64, :, :],
                                                     inv[:])
                                nc.sync.dma_start(aout[pair][64:128, m1],
                                                  tmpb[:])

                    def outproj_item(mt):
                        msl = slice(mt * 128, (mt + 1) * 128)
                        ps = scp.tile([128, 2, 512], F32, tag="sc",
                                      name="ops")
                        for dk in range(PAIRS):
                            for nch in range(2):
                                nc.tensor.matmul(
                                    ps[:, nch, :],
                                    aout[dk][:, msl],
                                    wo_sb[:, dk, nch * 512:(nch + 1) * 512],
                                    start=(dk == 0),
                                    stop=(dk == PAIRS - 1),
                                )
                        ost = ostp.tile([128, E], F32, tag="ost")
                        nc.vector.tensor_add(ost[:], ps[:], bo_bc[:])
                        nc.sync.dma_start(out_d[msl, :], ost[:])

                    qproj_x(0)
                    for nt in range(PAIRS):
                        qproj_group(0, nt)
                    for mh in range(NMH):
                        for pair in range(PAIRS):
                            work = []
                            if mh == 0:
                                if pair == 0:
                                    qproj_x(1)
                                work.append(lambda nt=pair: qproj_group(1, nt))
                            else:
                                for mt in (2 * pair, 2 * pair + 1):
                                    work.append(
                                        lambda mtt=mt: outproj_item(mtt))
                            attention(mh, pair, work)
                    for mt in range(8, 16):
                        outproj_item(mt)

    n = _dedup_ldweights(nc)
    print(f"dedup_ldweights removed {n}")
    return nc


def kernel(**inputs):
    query = np.asarray(inputs["query"], np.float32)
    key = np.asarray(inputs["key"], np.float32)
    value = np.asarray(inputs["value"], np.float32)
    Wq = np.asarray(inputs["Wq"], np.float32)
    bq = np.asarray(inputs["bq"], np.float32)
    Wk = np.asarray(inputs["Wk"], np.float32)
    bk = np.asarray(inputs["bk"], np.float32)
    Wv = np.asarray(inputs["Wv"], np.float32)
    bv = np.asarray(inputs["bv"], np.float32)
    Wo = np.asarray(inputs["Wo"], np.float32)
    bo = np.asarray(inputs["bo"], np.float32)

    nc = build_nc()

    in_maps = []
    for c in range(8):
        b, hh = c // 2, c % 2
        hs = slice(hh * HS, (hh + 1) * HS)
        def prep(a):
            a = np.ascontiguousarray(a)
            return a.astype(np.float16) if PROJ_F16 else round_fp32r(a)

        bo_eff = bo * 0.5 + Wo[:, hs] @ bv[hs]
        in_maps.append({
            "xq_t": prep(query[b].T),
            "xk_t": prep(key[b].T),
            "xv_t": prep(value[b].T),
            "wq_t": prep(Wq[hs, :].T),
            "wk_t": prep(Wk[hs, :].T),
            "wv_t": prep(Wv[hs, :].T),
            "wo_t": np.ascontiguousarray(Wo[:, hs].T).astype(np.float16),
            "bq": np.ascontiguousarray(bq[hs]),
            "bo_row": bo_eff.reshape(1, E).astype(np.float32),
        })

    from concourse.bass_utils import run_bass_kernel_spmd
    nc.finalize()
    r = run_bass_kernel_spmd(nc, in_maps, core_ids=list(range(8)))
    globals()["LAST_RUN"] = r
    outs = [r.results[c]["out_partial"] for c in range(8)]
    return np.stack([outs[2 * b] + outs[2 * b + 1] for b in range(B)])



# revision 22
# speedup vs baseline: 1.2750x; 1.2750x over previous
"""MultiHeadAttention TRN2 Bass kernel (nn_MultiHeadAttention, B=4 S=2048 E=1024 H=16).

Sharding over 8 NeuronCores: core c -> (batch b = c//2, head-half hh = c%2).
Each core computes, for its batch and its 8 heads: the Q/K/V projections,
attention, and a partial out-projection over its 512 "dk" dims with an
effective bias added; the host sums the two partials per batch (Megatron
tensor-parallel with the all-reduce replaced by a host-side pair sum).

Bias algebra (saves PE work):
  - bk is dropped entirely: k += bk shifts every score row by a constant
    (q_i . bk independent of key j) which cancels in softmax.
  - bv is folded host-side: (P @ (V + 1 bv^T))/den = PV/den + bv, so
    bo_eff = bo/2 + Wo[:, hs] @ bv[hs] and no bias is applied in V-proj.
  - bo_eff is added by the DVE during the out-proj PSUM->SBUF move from a
    partition-broadcast SBUF tile (no K=1 ones matmuls on the PE).

LDWEIGHTS reduction (PE was ~35% weight-load overhead):
  - K-proj streams each weight tile over two 512-token chunks; the second
    (identical) legalized LDWEIGHTS is removed by _dedup_ldweights.
  - Scores: one explicit full-array 128-row ldweights covers BOTH 64-row
    band loads (kt head A on PE rows 0-63, head B on 64-127);
    _dedup_ldweights removes the two covered band loads.
  - Out-proj runs per m-tile with both 512-col chunks per aout stationary.

The attention pipeline itself is deliberately the PE-heavy groups-of-3
structure: per 512-col chunk the PE (scores+PV, 2x512cy @2.4GHz) and ACT
(exp, 512cy @1.2GHz) are exactly balanced, and the PE drops to a lower
p-state (1.2GHz) if it ever micro-idles, so the schedule must keep the PE
locally over-subscribed (weight loads + interleaved Q/out-proj work).

Measured: ~412us/core across 8 cores, rel err ~5.4e-4.
"""

import numpy as np

import concourse.bass as bass
import concourse.mybir as mybir
import concourse.tile as tile
from concourse import bacc

F32 = mybir.dt.float32
F16 = mybir.dt.float16
AF = mybir.ActivationFunctionType

B, S, E, H, D = 4, 2048, 1024, 16, 64
HS = 512            # dims per core (8 heads)
PAIRS = 4           # head pairs per core
MC = 512            # m1 chunk
NMC = S // MC       # 4
NKT = E // 128      # 8 contraction chunks for projections
NMT = S // 128      # 16 m2 tiles
GROUPS = [3, 3, 3, 3, 3, 1]   # m2-tile grouping for ACT exp ops


def _ap_key(w):
    return (w.memref, w.offset, tuple(tuple(p) for p in w.ap), w.dtype)


def _dedup_ldweights(nc):
    """Remove redundant InstLdweights.

    Engines execute their instructions in block order, so after a load the
    PE array keeps those weights until the next load.  A load L is
    redundant when the previous surviving load F (with only Matmults, which
    don't disturb the array, in between on the PE) satisfies either:
      - identical AP (same memref/offset/pattern), or
      - F is a full 128-row load and L is a 64-row band of it at the
        matching tile_position row (0 -> same offset, 64 -> offset plus 64
        partition strides).
    Sync info of a removed load moves to the next kept instruction.
    """
    removed = 0
    passthrough = ("TensorCopy", "TensorScalarPtr", "TensorTensor",
                   "Activation", "DMACopy", "Memset", "ISA",
                   "EventSemaphore", "TensorReduce", "Iota", "TensorScalar")
    for fn in nc.m.functions:
        for blk in fn.blocks:
            insts = list(blk.instructions)
            keep = []
            last = None          # (memref, offset, ap, dtype) of live load
            pending_sync = None
            for i in insts:
                drop = False
                if i.opcode == "Ldweights":
                    w = i.ins[0]
                    key = _ap_key(w)
                    if last is not None and key == last:
                        drop = True
                    elif last is not None and last[0] == key[0]:
                        mref, off, ap, dt_ = last
                        if (len(ap) == 2 and len(key[2]) == 2
                                and dt_ == key[3]
                                and ap[0][1] == 128 and key[2][0][1] == 64
                                and ap[0][0] == key[2][0][0]
                                and ap[1] == key[2][1]):
                            tp = getattr(i, "tile_position", None)
                            stride = ap[0][0]
                            if tp is not None and (
                                (tp[0] == 0 and key[1] == off)
                                or (tp[0] == 64 and key[1] == off
                                    + 64 * stride)):
                                drop = True
                    if not drop:
                        last = key
                elif i.opcode == "Matmult":
                    pass  # uses loaded weights, doesn't clobber them
                elif i.opcode in passthrough:
                    pass  # other engines don't touch the PE array
                else:
                    last = None  # control flow / drains: be conservative
                if drop:
                    si = i.sync_info
                    if si is not None and (si.on_wait or si.on_update):
                        if pending_sync is None:
                            pending_sync = si
                        else:
                            for w_ in si.on_wait:
                                pending_sync.on_wait.append(w_)
                            for u_ in si.on_update:
                                pending_sync.on_update.append(u_)
                    removed += 1
                    continue
                if pending_sync is not None:
                    si = i.sync_info
                    if si is None:
                        i.sync_info = pending_sync
                    else:
                        for w_ in pending_sync.on_wait:
                            si.on_wait.append(w_)
                        for u_ in pending_sync.on_update:
                            si.on_update.append(u_)
                    pending_sync = None
                keep.append(i)
            if len(keep) != len(insts):
                blk.instructions = keep
    return removed


def build_nc():
    nc = bacc.Bacc()

    xq_d = nc.dram_tensor("xq_t", [E, S], F16, kind="ExternalInput")
    xk_d = nc.dram_tensor("xk_t", [E, S], F16, kind="ExternalInput")
    xv_d = nc.dram_tensor("xv_t", [E, S], F16, kind="ExternalInput")
    wq_d = nc.dram_tensor("wq_t", [E, HS], F16, kind="ExternalInput")
    wk_d = nc.dram_tensor("wk_t", [E, HS], F16, kind="ExternalInput")
    wv_d = nc.dram_tensor("wv_t", [E, HS], F16, kind="ExternalInput")
    wo_d = nc.dram_tensor("wo_t", [HS, E], F16, kind="ExternalInput")
    bq_d = nc.dram_tensor("bq", [HS], F32, kind="ExternalInput")
    bo_d = nc.dram_tensor("bo_row", [1, E], F32, kind="ExternalInput")

    out_d = nc.dram_tensor("out_partial", [S, E], F32, kind="ExternalOutput")
    scratch_d = nc.dram_tensor("scratch_v5", [NMC, PAIRS, 2, MC], F32)

    def bcast_ap(row_ap, n):
        return bass.AP(tensor=row_ap.tensor, offset=row_ap.offset,
                       ap=[[0, n]] + list(row_ap.ap[1:]))

    with tile.TileContext(nc) as tc:
        with (
            tc.tile_pool(name="const", bufs=1) as const,
            tc.tile_pool(name="qkv", bufs=1) as qkv,
            tc.tile_pool(name="aout", bufs=1) as aoutp,
        ):
            bq_sb = const.tile([128, PAIRS], F32)
            nc.sync.dma_start(bq_sb[:], bq_d.rearrange("(t p) -> p t", p=128))
            bo_bc = const.tile([128, E], F32)
            nc.sync.dma_start(bo_bc[:], bcast_ap(bo_d[:], 128))

            qt_all = qkv.tile([128, PAIRS, S], F16, tag="qt")
            kt_all = qkv.tile([128, PAIRS, S], F16, tag="kt")
            v_all = qkv.tile([128, NMT, 8, 65], F16, tag="v")
            nc.vector.memset(v_all[:, :, :, 64], 1.0)

            aout = [aoutp.tile([128, S], F16, name=f"aout{p}", tag=f"ao{p}")
                    for p in range(PAIRS)]

            with (
                tc.tile_pool(name="w", bufs=2) as wpool,
                tc.tile_pool(name="x", bufs=2) as xpool,
            ):
                # ======== K and V projections (own PSUM scope) ========
                with tc.tile_pool(name="pp", bufs=2,
                                  space=bass.MemorySpace.PSUM) as pp:
                    wk_sb = wpool.tile([128, NKT, HS], F16, tag="w")
                    wk_r = wk_d.rearrange("(kc p) n -> p kc n", p=128)
                    # split the first weight/x DMAs per-kc so the first
                    # matmul starts after ~1/8 of the data has landed
                    for kc in range(NKT):
                        nc.sync.dma_start(wk_sb[:, kc, :], wk_r[:, kc, :])

                    for mch in range(NMC // 2):
                        msl = slice(mch * 2 * MC, (mch + 1) * 2 * MC)
                        x_t = xpool.tile([128, NKT, 2 * MC], F16, tag="x")
                        xk_r = xk_d.rearrange("(kc p) m -> p kc m", p=128)
                        if mch == 0:
                            for kc in range(NKT):
                                nc.sync.dma_start(x_t[:, kc, :],
                                                  xk_r[:, kc, msl])
                        else:
                            nc.sync.dma_start(x_t[:], xk_r[:, :, msl])
                        for nt in range(PAIRS):
                            ps = pp.tile([128, 2, MC], F32, tag="ppk")
                            for kc in range(NKT):
                                for h in range(2):
                                    nc.tensor.matmul(
                                        ps[:, h, :],
                                        wk_sb[:, kc, nt * 128:(nt + 1) * 128],
                                        x_t[:, kc, h * MC:(h + 1) * MC],
                                        start=(kc == 0),
                                        stop=(kc == NKT - 1),
                                    )
                            nc.vector.tensor_copy(
                                kt_all[:, nt, msl], ps[:])

                    wv_sb = wpool.tile([128, NKT, HS], F16, tag="w")
                    nc.sync.dma_start(
                        wv_sb[:], wv_d.rearrange("(kc p) n -> p kc n", p=128))
                    for mch in range(NMC // 2):
                        msl = slice(mch * 2 * MC, (mch + 1) * 2 * MC)
                        x_t = xpool.tile([128, NKT, 2 * MC], F16, tag="x")
                        nc.sync.dma_start(
                            x_t[:],
                            xv_d.rearrange("(kc p) m -> p kc m", p=128)[
                                :, :, msl
                            ],
                        )
                        for mt_l in range(2 * MC // 128):
                            mt = mch * (2 * MC // 128) + mt_l
                            ps = pp.tile([128, HS], F32, tag="ppv", bufs=4)
                            for kc in range(NKT):
                                nc.tensor.matmul(
                                    ps[:],
                                    x_t[:, kc, mt_l * 128:(mt_l + 1) * 128],
                                    wv_sb[:, kc, :],
                                    start=(kc == 0),
                                    stop=(kc == NKT - 1),
                                )
                            nc.vector.tensor_copy(
                                v_all[:, mt, :, 0:64],
                                ps[:].rearrange("p (h c) -> p h c", c=64),
                            )

                # ======== Q-proj interleaved with attention + out-proj ====
                wq_sb = wpool.tile([128, NKT, HS], F16, tag="w")
                nc.sync.dma_start(
                    wq_sb[:], wq_d.rearrange("(kc p) n -> p kc n", p=128))

                with (
                    tc.tile_pool(name="wo", bufs=1) as wop,
                    tc.tile_pool(name="pt", bufs=4) as ptp,
                    tc.tile_pool(name="msc", bufs=2) as msc,
                    tc.tile_pool(name="ost", bufs=4) as ostp,
                    tc.tile_pool(name="sc", bufs=2,
                                 space=bass.MemorySpace.PSUM) as scp,
                    tc.tile_pool(name="pv", bufs=2,
                                 space=bass.MemorySpace.PSUM) as pvp,
                ):
                    wo_sb = wop.tile([128, PAIRS, E], F16, tag="wo")
                    nc.sync.dma_start(
                        wo_sb[:], wo_d.rearrange("(dk p) n -> p dk n", p=128))

                    qx = {}

                    def qproj_x(mc):
                        x_t = xpool.tile([128, NKT, MC], F16, tag="x",
                                         name=f"xq{mc}")
                        nc.sync.dma_start(
                            x_t[:],
                            xq_d.rearrange("(kc p) m -> p kc m", p=128)[
                                :, :, mc * MC:(mc + 1) * MC
                            ],
                        )
                        qx[mc] = x_t

                    def qproj_group(mc, nt):
                        x_t = qx[mc]
                        ps = pvp.tile([128, MC], F32, tag="pv")
                        for kc in range(NKT):
                            nc.tensor.matmul(
                                ps[:],
                                wq_sb[:, kc, nt * 128:(nt + 1) * 128],
                                x_t[:, kc, :],
                                start=(kc == 0),
                                stop=(kc == NKT - 1),
                            )
                        nc.vector.tensor_scalar_add(
                            qt_all[:, nt, mc * MC:(mc + 1) * MC],
                            ps[:],
                            bq_sb[:, nt:nt + 1],
                        )

                    def attention(mc, pair, extra=()):
                        extra = list(extra)
                        m1 = slice(mc * MC, (mc + 1) * MC)
                        pvt = [pvp.tile([128, MC], F32, name=f"pv{h}",
                                        tag="pv") for h in range(2)]
                        mt0 = 0
                        for gidx, gsize in enumerate(GROUPS):
                            scA = scp.tile([128, 3, MC], F32, tag="sc")
                            scB = scp.tile([128, 3, MC], F32, tag="sc")
                            for gi in range(gsize):
                                t = mt0 + gi
                                m2 = slice(t * 128, (t + 1) * 128)
                                # one full-array load covers both band
                                # loads below (they are deduped away)
                                nc.tensor.ldweights(
                                    kt_all[:, pair, m2],
                                    tile_position=(0, 0))
                                nc.tensor.matmul(
                                    scA[:, gi, :],
                                    kt_all[0:64, pair, m2],
                                    qt_all[0:64, pair, m1],
                                    start=True, stop=True,
                                    tile_position=(0, 0),
                                )
                                nc.tensor.matmul(
                                    scB[:, gi, :],
                                    kt_all[64:128, pair, m2],
                                    qt_all[64:128, pair, m1],
                                    start=True, stop=True,
                                    tile_position=(64, 0),
                                )
                            ptA = ptp.tile([128, 3, MC], F16, tag="pt")
                            ptB = ptp.tile([128, 3, MC], F16, tag="pt")
                            nc.scalar.activation(
                                ptA[:, 0:gsize, :], scA[:, 0:gsize, :],
                                AF.Exp, scale=0.125,
                            )
                            nc.scalar.activation(
                                ptB[:, 0:gsize, :], scB[:, 0:gsize, :],
                                AF.Exp, scale=0.125,
                            )
                            for gi in range(gsize):
                                t = mt0 + gi
                                nc.tensor.matmul(
                                    pvt[0][0:65, :],
                                    v_all[:, t, 2 * pair, :],
                                    ptA[:, gi, :],
                                    start=(t == 0), stop=(t == NMT - 1),
                                )
                                nc.tensor.matmul(
                                    pvt[1][0:65, :],
                                    v_all[:, t, 2 * pair + 1, :],
                                    ptB[:, gi, :],
                                    start=(t == 0), stop=(t == NMT - 1),
                                )
                            mt0 += gsize
                            # interleave one spread-work item (out-proj or
                            # Q-proj group) between m2-groups so PE and ACT
                            # stay fed through pair and chunk boundaries
                            if gidx % 2 == 1 and extra:
                                extra.pop(0)()
                        while extra:
                            extra.pop(0)()

                        # normalize: out_h = pv[0:64] / pv[64].  Copy
                        # PSUM->SBUF immediately (frees the pv bank for the
                        # next pair), then divide from the SBUF copy.
                        for h in range(2):
                            pvs = msc.tile([128, MC], F32, name=f"pvs{h}",
                                           tag="pvs")
                            nc.vector.tensor_copy(pvs[0:65, :],
                                                  pvt[h][0:65, :])
                            srow_dram = scratch_d[mc:mc + 1, pair, h, :]
                            nc.sync.dma_start(srow_dram, pvs[64:65, :])
                            bc = msc.tile([64, MC], F32, tag="bc")
                            nc.sync.dma_start(bc[:], bcast_ap(srow_dram, 64))
                            inv = msc.tile([64, MC], F32, tag="inv")
                            nc.vector.reciprocal_approx_fast(inv[:], bc[:])
                            if h == 0:
                                nc.vector.tensor_mul(
                                    aout[pair][0:64, m1], pvs[0:64, :],
                                    inv[:],
                                )
                            else:
                                tmpb = msc.tile([64, MC], F16, tag="tmpb")
                                nc.vector.tensor_mul(tmpb[:], pvs[0:64, :],
                                                     inv[:])
                                nc.sync.dma_start(aout[pair][64:128, m1],
                                                  tmpb[:])

                    def outproj_item(mt):
                        # one m-tile, both 512-col chunks: each aout
                        # stationary is loaded once (second load deduped)
                        msl = slice(mt * 128, (mt + 1) * 128)
                        ps = [pvp.tile([128, 512], F32, tag="pv",
                                       name=f"op{n}") for n in range(2)]
                        for dk in range(PAIRS):
                            for nch in range(2):
                                nc.tensor.matmul(
                                    ps[nch][:],
                                    aout[dk][:, msl],
                                    wo_sb[:, dk,
                                          nch * 512:(nch + 1) * 512],
                                    start=(dk == 0),
                                    stop=(dk == PAIRS - 1),
                                )
                        ost = ostp.tile([128, E], F32, tag="ost")
                        for nch in range(2):
                            nc.vector.tensor_add(
                                ost[:, nch * 512:(nch + 1) * 512],
                                ps[nch][:],
                                bo_bc[:, nch * 512:(nch + 1) * 512])
                        nc.sync.dma_start(out_d[msl, :], ost[:])

                    qproj_x(0)
                    for nt in range(PAIRS):
                        qproj_group(0, nt)
                    for mc in range(NMC):
                        for pair in range(PAIRS):
                            if pair == 0 and mc + 1 < NMC:
                                qproj_x(mc + 1)
                            work = []
                            if mc >= 1:
                                mt = (mc - 1) * (MC // 128) + pair
                                work.append(lambda mtt=mt:
                                            outproj_item(mtt))
                            if mc + 1 < NMC:
                                work.append(
                                    lambda mcc=mc + 1, nt=pair:
                                    qproj_group(mcc, nt))
                            attention(mc, pair, work)
                    for mt in range(12, 16):
                        outproj_item(mt)

    n = _dedup_ldweights(nc)
    print(f"dedup_ldweights removed {n}")
    return nc


def kernel(**inputs):
    query = np.asarray(inputs["query"], np.float32)
    key = np.asarray(inputs["key"], np.float32)
    value = np.asarray(inputs["value"], np.float32)
    Wq = np.asarray(inputs["Wq"], np.float32)
    bq = np.asarray(inputs["bq"], np.float32)
    Wk = np.asarray(inputs["Wk"], np.float32)
    Wv = np.asarray(inputs["Wv"], np.float32)
    bv = np.asarray(inputs["bv"], np.float32)
    Wo = np.asarray(inputs["Wo"], np.float32)
    bo = np.asarray(inputs["bo"], np.float32)

    nc = build_nc()

    in_maps = []
    for c in range(8):
        b, hh = c // 2, c % 2
        hs = slice(hh * HS, (hh + 1) * HS)

        def prep(a):
            return np.ascontiguousarray(a).astype(np.float16)

        bo_eff = bo * 0.5 + Wo[:, hs] @ bv[hs]
        in_maps.append({
            "xq_t": prep(query[b].T),
            "xk_t": prep(key[b].T),
            "xv_t": prep(value[b].T),
            "wq_t": prep(Wq[hs, :].T),
            "wk_t": prep(Wk[hs, :].T),
            "wv_t": prep(Wv[hs, :].T),
            "wo_t": prep(Wo[:, hs].T),
            "bq": np.ascontiguousarray(bq[hs]),
            "bo_row": bo_eff.reshape(1, E).astype(np.float32),
        })

    from concourse.bass_utils import run_bass_kernel_spmd
    nc.finalize()
    r = run_bass_kernel_spmd(nc, in_maps, core_ids=list(range(8)))
    globals()["LAST_RUN"] = r
    outs = [r.results[c]["out_partial"] for c in range(8)]
    return np.stack([outs[2 * b] + outs[2 * b + 1] for b in range(B)])


# revision 24
# speedup vs baseline: 1.2852x; 1.0080x over previous
"""MultiHeadAttention TRN2 Bass kernel (nn_MultiHeadAttention, B=4 S=2048 E=1024 H=16).

Sharding over 8 NeuronCores: core c -> (batch b = c//2, head-half hh = c%2).
Each core computes, for its batch and its 8 heads: the Q/K/V projections,
attention, and a partial out-projection over its 512 "dk" dims with an
effective bias added; the host sums the two partials per batch (Megatron
tensor-parallel with the all-reduce replaced by a host-side pair sum).

Bias algebra (saves PE work):
  - bk is dropped entirely: k += bk shifts every score row by a constant
    (q_i . bk independent of key j) which cancels in softmax.
  - bv is folded host-side: (P @ (V + 1 bv^T))/den = PV/den + bv, so
    bo_eff = bo/2 + Wo[:, hs] @ bv[hs] and no bias is applied in V-proj.
  - bo_eff is added by the DVE during the out-proj PSUM->SBUF move from a
    partition-broadcast SBUF tile (no K=1 ones matmuls on the PE).

LDWEIGHTS reduction (PE was ~35% weight-load overhead):
  - K-proj streams each weight tile over two 512-token chunks; the second
    (identical) legalized LDWEIGHTS is removed by _dedup_ldweights.
  - Scores: one explicit full-array 128-row ldweights covers BOTH 64-row
    band loads (kt head A on PE rows 0-63, head B on 64-127);
    _dedup_ldweights removes the two covered band loads.
  - Out-proj runs per m-tile with both 512-col chunks per aout stationary.

The attention pipeline itself is deliberately the PE-heavy groups-of-3
structure: per 512-col chunk the PE (scores+PV, 2x512cy @2.4GHz) and ACT
(exp, 512cy @1.2GHz) are exactly balanced, and the PE drops to a lower
p-state (1.2GHz) if it ever micro-idles, so the schedule must keep the PE
locally over-subscribed (weight loads + interleaved Q/out-proj work).

Measured: ~412us/core across 8 cores, rel err ~5.4e-4.
"""

import numpy as np

import concourse.bass as bass
import concourse.mybir as mybir
import concourse.tile as tile
from concourse import bacc

F32 = mybir.dt.float32
F16 = mybir.dt.float16
AF = mybir.ActivationFunctionType

B, S, E, H, D = 4, 2048, 1024, 16, 64
HS = 512            # dims per core (8 heads)
PAIRS = 4           # head pairs per core
MC = 512            # m1 chunk
NMC = S // MC       # 4
NKT = E // 128      # 8 contraction chunks for projections
NMT = S // 128      # 16 m2 tiles
GROUPS = [3, 3, 3, 3, 3, 1]   # m2-tile grouping for ACT exp ops


def _ap_key(w):
    return (w.memref, w.offset, tuple(tuple(p) for p in w.ap), w.dtype)


def _dedup_ldweights(nc):
    """Remove redundant InstLdweights.

    Engines execute their instructions in block order, so after a load the
    PE array keeps those weights until the next load.  A load L is
    redundant when the previous surviving load F (with only Matmults, which
    don't disturb the array, in between on the PE) satisfies either:
      - identical AP (same memref/offset/pattern), or
      - F is a full 128-row load and L is a 64-row band of it at the
        matching tile_position row (0 -> same offset, 64 -> offset plus 64
        partition strides).
    Additionally, a 64-row band load at tile_position (0,0) directly
    followed (on the PE) by its sibling band load at (64,0) of the same
    tensor is merged: the first is widened in place to a 128-row full-array
    load and the second is dropped.

    Sync info of a removed load moves to the next kept instruction.
    """
    removed = 0
    passthrough = ("TensorCopy", "TensorScalarPtr", "TensorTensor",
                   "Activation", "DMACopy", "Memset", "ISA",
                   "EventSemaphore", "TensorReduce", "Iota", "TensorScalar")
    for fn in nc.m.functions:
        for blk in fn.blocks:
            insts = list(blk.instructions)
            keep = []
            last = None          # (memref, offset, ap, dtype) of live load
            last_inst = None     # the kept Ldweights object for widening
            pending_sync = None
            for i in insts:
                drop = False
                if i.opcode == "Ldweights":
                    w = i.ins[0]
                    key = _ap_key(w)
                    if last is not None and key == last:
                        drop = True
                    elif last is not None and last[0] == key[0]:
                        mref, off, ap, dt_ = last
                        stride = ap[0][0] if ap else None
                        tp = getattr(i, "tile_position", None)
                        ltp = getattr(last_inst, "tile_position", None) \
                            if last_inst is not None else None
                        if (len(ap) == 2 and len(key[2]) == 2
                                and dt_ == key[3]
                                and ap[1] == key[2][1]
                                and ap[0][0] == key[2][0][0]):
                            if (ap[0][1] == 128 and key[2][0][1] == 64
                                    and tp is not None
                                    and ((tp[0] == 0 and key[1] == off)
                                         or (tp[0] == 64 and key[1] == off
                                             + 64 * stride))):
                                # covered by an existing full load
                                drop = True
                            elif (ap[0][1] == 64 and key[2][0][1] == 64
                                    and ltp is not None and ltp[0] == 0
                                    and tp is not None and tp[0] == 64
                                    and key[1] == off + 64 * stride):
                                # widen the previous top-band load to a
                                # full-array load; drop this bottom one
                                lw = last_inst.ins[0]
                                lw.ap = [[stride, 128], list(ap[1])]
                                ts = getattr(last_inst, "tile_size", None)
                                if ts is not None:
                                    last_inst.tile_size = (128, ts[1])
                                last = (mref, off,
                                        ((stride, 128), tuple(ap[1])),
                                        dt_)
                                drop = True
                    if not drop:
                        last = key
                        last_inst = i
                elif i.opcode == "Matmult":
                    pass  # uses loaded weights, doesn't clobber them
                elif i.opcode in passthrough:
                    pass  # other engines don't touch the PE array
                else:
                    last = None  # control flow / drains: be conservative
                    last_inst = None
                if drop:
                    si = i.sync_info
                    if si is not None and (si.on_wait or si.on_update):
                        if pending_sync is None:
                            pending_sync = si
                        else:
                            for w_ in si.on_wait:
                                pending_sync.on_wait.append(w_)
                            for u_ in si.on_update:
                                pending_sync.on_update.append(u_)
                    removed += 1
                    continue
                if pending_sync is not None:
                    si = i.sync_info
                    if si is None:
                        i.sync_info = pending_sync
                    else:
                        for w_ in pending_sync.on_wait:
                            si.on_wait.append(w_)
                        for u_ in pending_sync.on_update:
                            si.on_update.append(u_)
                    pending_sync = None
                keep.append(i)
            if len(keep) != len(insts):
                blk.instructions = keep
    return removed


def build_nc():
    nc = bacc.Bacc()

    xq_d = nc.dram_tensor("xq_t", [E, S], F16, kind="ExternalInput")
    xk_d = nc.dram_tensor("xk_t", [E, S], F16, kind="ExternalInput")
    xv_d = nc.dram_tensor("xv_t", [E, S], F16, kind="ExternalInput")
    wq_d = nc.dram_tensor("wq_t", [E, HS], F16, kind="ExternalInput")
    wk_d = nc.dram_tensor("wk_t", [E, HS], F16, kind="ExternalInput")
    wv_d = nc.dram_tensor("wv_t", [E, HS], F16, kind="ExternalInput")
    wo_d = nc.dram_tensor("wo_t", [HS, E], F16, kind="ExternalInput")
    bq_d = nc.dram_tensor("bq", [HS], F32, kind="ExternalInput")
    bo_d = nc.dram_tensor("bo_row", [1, E], F32, kind="ExternalInput")

    out_d = nc.dram_tensor("out_partial", [S, E], F32, kind="ExternalOutput")
    scratch_d = nc.dram_tensor("scratch_v5", [NMC, PAIRS, 2, MC], F32)

    def bcast_ap(row_ap, n):
        return bass.AP(tensor=row_ap.tensor, offset=row_ap.offset,
                       ap=[[0, n]] + list(row_ap.ap[1:]))

    with tile.TileContext(nc) as tc:
        with (
            tc.tile_pool(name="const", bufs=1) as const,
            tc.tile_pool(name="qkv", bufs=1) as qkv,
            tc.tile_pool(name="aout", bufs=1) as aoutp,
        ):
            bq_sb = const.tile([128, PAIRS], F32)
            nc.sync.dma_start(bq_sb[:], bq_d.rearrange("(t p) -> p t", p=128))
            bo_bc = const.tile([128, E], F32)
            nc.sync.dma_start(bo_bc[:], bcast_ap(bo_d[:], 128))

            qt_all = qkv.tile([128, PAIRS, S], F16, tag="qt")
            kt_all = qkv.tile([128, PAIRS, S], F16, tag="kt")
            v_all = qkv.tile([128, NMT, 8, 65], F16, tag="v")
            nc.vector.memset(v_all[:, :, :, 64], 1.0)

            aout = [aoutp.tile([128, S], F16, name=f"aout{p}", tag=f"ao{p}")
                    for p in range(PAIRS)]

            with (
                tc.tile_pool(name="w", bufs=2) as wpool,
                tc.tile_pool(name="x", bufs=2) as xpool,
            ):
                # ======== K and V projections (own PSUM scope) ========
                with tc.tile_pool(name="pp", bufs=2,
                                  space=bass.MemorySpace.PSUM) as pp:
                    wk_sb = wpool.tile([128, NKT, HS], F16, tag="w")
                    wk_r = wk_d.rearrange("(kc p) n -> p kc n", p=128)
                    # split the first weight/x DMAs per-kc so the first
                    # matmul starts after ~1/8 of the data has landed
                    for kc in range(NKT):
                        nc.sync.dma_start(wk_sb[:, kc, :], wk_r[:, kc, :])

                    for mch in range(NMC // 2):
                        msl = slice(mch * 2 * MC, (mch + 1) * 2 * MC)
                        x_t = xpool.tile([128, NKT, 2 * MC], F16, tag="x")
                        xk_r = xk_d.rearrange("(kc p) m -> p kc m", p=128)
                        if mch == 0:
                            for kc in range(NKT):
                                nc.sync.dma_start(x_t[:, kc, :],
                                                  xk_r[:, kc, msl])
                        else:
                            nc.sync.dma_start(x_t[:], xk_r[:, :, msl])
                        for nt in range(PAIRS):
                            ps = pp.tile([128, 2, MC], F32, tag="ppk")
                            for kc in range(NKT):
                                for h in range(2):
                                    nc.tensor.matmul(
                                        ps[:, h, :],
                                        wk_sb[:, kc, nt * 128:(nt + 1) * 128],
                                        x_t[:, kc, h * MC:(h + 1) * MC],
                                        start=(kc == 0),
                                        stop=(kc == NKT - 1),
                                    )
                            nc.vector.tensor_copy(
                                kt_all[:, nt, msl], ps[:])

                    wv_sb = wpool.tile([128, NKT, HS], F16, tag="w")
                    nc.sync.dma_start(
                        wv_sb[:], wv_d.rearrange("(kc p) n -> p kc n", p=128))
                    for mch in range(NMC // 2):
                        msl = slice(mch * 2 * MC, (mch + 1) * 2 * MC)
                        x_t = xpool.tile([128, NKT, 2 * MC], F16, tag="x")
                        nc.sync.dma_start(
                            x_t[:],
                            xv_d.rearrange("(kc p) m -> p kc m", p=128)[
                                :, :, msl
                            ],
                        )
                        for mt_l in range(2 * MC // 128):
                            mt = mch * (2 * MC // 128) + mt_l
                            ps = pp.tile([128, HS], F32, tag="ppv", bufs=4)
                            for kc in range(NKT):
                                nc.tensor.matmul(
                                    ps[:],
                                    x_t[:, kc, mt_l * 128:(mt_l + 1) * 128],
                                    wv_sb[:, kc, :],
                                    start=(kc == 0),
                                    stop=(kc == NKT - 1),
                                )
                            nc.vector.tensor_copy(
                                v_all[:, mt, :, 0:64],
                                ps[:].rearrange("p (h c) -> p h c", c=64),
                            )

                # ======== Q-proj interleaved with attention + out-proj ====
                wq_sb = wpool.tile([128, NKT, HS], F16, tag="w")
                nc.sync.dma_start(
                    wq_sb[:], wq_d.rearrange("(kc p) n -> p kc n", p=128))

                with (
                    tc.tile_pool(name="wo", bufs=1) as wop,
                    tc.tile_pool(name="pt", bufs=4) as ptp,
                    tc.tile_pool(name="msc", bufs=2) as msc,
                    tc.tile_pool(name="ost", bufs=4) as ostp,
                    tc.tile_pool(name="sc", bufs=2,
                                 space=bass.MemorySpace.PSUM) as scp,
                    tc.tile_pool(name="pv", bufs=2,
                                 space=bass.MemorySpace.PSUM) as pvp,
                ):
                    wo_sb = wop.tile([128, PAIRS, E], F16, tag="wo")
                    nc.sync.dma_start(
                        wo_sb[:], wo_d.rearrange("(dk p) n -> p dk n", p=128))

                    qx = {}

                    def qproj_x(mc):
                        x_t = xpool.tile([128, NKT, MC], F16, tag="x",
                                         name=f"xq{mc}")
                        nc.sync.dma_start(
                            x_t[:],
                            xq_d.rearrange("(kc p) m -> p kc m", p=128)[
                                :, :, mc * MC:(mc + 1) * MC
                            ],
                        )
                        qx[mc] = x_t

                    def qproj_group(mc, nt):
                        x_t = qx[mc]
                        ps = pvp.tile([128, MC], F32, tag="pv")
                        for kc in range(NKT):
                            nc.tensor.matmul(
                                ps[:],
                                wq_sb[:, kc, nt * 128:(nt + 1) * 128],
                                x_t[:, kc, :],
                                start=(kc == 0),
                                stop=(kc == NKT - 1),
                            )
                        nc.vector.tensor_scalar_add(
                            qt_all[:, nt, mc * MC:(mc + 1) * MC],
                            ps[:],
                            bq_sb[:, nt:nt + 1],
                        )

                    def attention(mc, pair, extra=()):
                        extra = list(extra)
                        m1 = slice(mc * MC, (mc + 1) * MC)
                        pvt = [pvp.tile([128, MC], F32, name=f"pv{h}",
                                        tag="pv") for h in range(2)]
                        mt0 = 0
                        for gidx, gsize in enumerate(GROUPS):
                            scA = scp.tile([128, 3, MC], F32, tag="sc")
                            scB = scp.tile([128, 3, MC], F32, tag="sc")
                            for gi in range(gsize):
                                t = mt0 + gi
                                m2 = slice(t * 128, (t + 1) * 128)
                                # the two band loads (rows 0-63 / 64-127)
                                # are merged into one 128-row load by
                                # _dedup_ldweights
                                nc.tensor.matmul(
                                    scA[:, gi, :],
                                    kt_all[0:64, pair, m2],
                                    qt_all[0:64, pair, m1],
                                    start=True, stop=True,
                                    tile_position=(0, 0),
                                )
                                nc.tensor.matmul(
                                    scB[:, gi, :],
                                    kt_all[64:128, pair, m2],
                                    qt_all[64:128, pair, m1],
                                    start=True, stop=True,
                                    tile_position=(64, 0),
                                )
                            ptA = ptp.tile([128, 3, MC], F16, tag="pt")
                            ptB = ptp.tile([128, 3, MC], F16, tag="pt")
                            nc.scalar.activation(
                                ptA[:, 0:gsize, :], scA[:, 0:gsize, :],
                                AF.Exp, scale=0.125,
                            )
                            nc.scalar.activation(
                                ptB[:, 0:gsize, :], scB[:, 0:gsize, :],
                                AF.Exp, scale=0.125,
                            )
                            for gi in range(gsize):
                                t = mt0 + gi
                                nc.tensor.matmul(
                                    pvt[0][0:65, :],
                                    v_all[:, t, 2 * pair, :],
                                    ptA[:, gi, :],
                                    start=(t == 0), stop=(t == NMT - 1),
                                )
                                nc.tensor.matmul(
                                    pvt[1][0:65, :],
                                    v_all[:, t, 2 * pair + 1, :],
                                    ptB[:, gi, :],
                                    start=(t == 0), stop=(t == NMT - 1),
                                )
                            mt0 += gsize
                            # interleave one spread-work item (out-proj or
                            # Q-proj group) between m2-groups so PE and ACT
                            # stay fed through pair and chunk boundaries
                            if gidx % 2 == 1 and extra:
                                extra.pop(0)()
                        while extra:
                            extra.pop(0)()

                        # normalize: out_h = pv[0:64] / pv[64].  Copy
                        # PSUM->SBUF immediately (frees the pv bank for the
                        # next pair), then divide from the SBUF copy.
                        for h in range(2):
                            pvs = msc.tile([128, MC], F32, name=f"pvs{h}",
                                           tag="pvs")
                            nc.vector.tensor_copy(pvs[0:65, :],
                                                  pvt[h][0:65, :])
                            srow_dram = scratch_d[mc:mc + 1, pair, h, :]
                            nc.sync.dma_start(srow_dram, pvs[64:65, :])
                            bc = msc.tile([64, MC], F32, tag="bc")
                            nc.sync.dma_start(bc[:], bcast_ap(srow_dram, 64))
                            inv = msc.tile([64, MC], F32, tag="inv")
                            nc.vector.reciprocal_approx_fast(inv[:], bc[:])
                            if h == 0:
                                nc.vector.tensor_mul(
                                    aout[pair][0:64, m1], pvs[0:64, :],
                                    inv[:],
                                )
                            else:
                                tmpb = msc.tile([64, MC], F16, tag="tmpb")
                                nc.vector.tensor_mul(tmpb[:], pvs[0:64, :],
                                                     inv[:])
                                nc.sync.dma_start(aout[pair][64:128, m1],
                                                  tmpb[:])

                    def outproj_item(mt):
                        # one m-tile, both 512-col chunks: each aout
                        # stationary is loaded once (second load deduped)
                        msl = slice(mt * 128, (mt + 1) * 128)
                        ps = [pvp.tile([128, 512], F32, tag="pv",
                                       name=f"op{n}") for n in range(2)]
                        for dk in range(PAIRS):
                            for nch in range(2):
                                nc.tensor.matmul(
                                    ps[nch][:],
                                    aout[dk][:, msl],
                                    wo_sb[:, dk,
                                          nch * 512:(nch + 1) * 512],
                                    start=(dk == 0),
                                    stop=(dk == PAIRS - 1),
                                )
                        ost = ostp.tile([128, E], F32, tag="ost")
                        for nch in range(2):
                            nc.vector.tensor_add(
                                ost[:, nch * 512:(nch + 1) * 512],
                                ps[nch][:],
                                bo_bc[:, nch * 512:(nch + 1) * 512])
                        nc.sync.dma_start(out_d[msl, :], ost[:])

                    qproj_x(0)
                    for nt in range(PAIRS):
                        qproj_group(0, nt)
                    for mc in range(NMC):
                        for pair in range(PAIRS):
                            if pair == 0 and mc + 1 < NMC:
                                qproj_x(mc + 1)
                            work = []
                            if mc >= 1:
                                mt = (mc - 1) * (MC // 128) + pair
                                work.append(lambda mtt=mt:
                                            outproj_item(mtt))
                            if mc + 1 < NMC:
                                work.append(
                                    lambda mcc=mc + 1, nt=pair:
                                    qproj_group(mcc, nt))
                            attention(mc, pair, work)
                    for mt in range(12, 16):
                        outproj_item(mt)

    n = _dedup_ldweights(nc)
    print(f"dedup_ldweights removed {n}")
    return nc


def kernel(**inputs):
    query = np.asarray(inputs["query"], np.float32)
    key = np.asarray(inputs["key"], np.float32)
    value = np.asarray(inputs["value"], np.float32)
    Wq = np.asarray(inputs["Wq"], np.float32)
    bq = np.asarray(inputs["bq"], np.float32)
    Wk = np.asarray(inputs["Wk"], np.float32)
    Wv = np.asarray(inputs["Wv"], np.float32)
    bv = np.asarray(inputs["bv"], np.float32)
    Wo = np.asarray(inputs["Wo"], np.float32)
    bo = np.asarray(inputs["bo"], np.float32)

    nc = build_nc()

    in_maps = []
    for c in range(8):
        b, hh = c // 2, c % 2
        hs = slice(hh * HS, (hh + 1) * HS)

        def prep(a):
            return np.ascontiguousarray(a).astype(np.float16)

        bo_eff = bo * 0.5 + Wo[:, hs] @ bv[hs]
        in_maps.append({
            "xq_t": prep(query[b].T),
            "xk_t": prep(key[b].T),
            "xv_t": prep(value[b].T),
            "wq_t": prep(Wq[hs, :].T),
            "wk_t": prep(Wk[hs, :].T),
            "wv_t": prep(Wv[hs, :].T),
            "wo_t": prep(Wo[:, hs].T),
            "bq": np.ascontiguousarray(bq[hs]),
            "bo_row": bo_eff.reshape(1, E).astype(np.float32),
        })

    from concourse.bass_utils import run_bass_kernel_spmd
    nc.finalize()
    r = run_bass_kernel_spmd(nc, in_maps, core_ids=list(range(8)))
    globals()["LAST_RUN"] = r
    outs = [r.results[c]["out_partial"] for c in range(8)]
    return np.stack([outs[2 * b] + outs[2 * b + 1] for b in range(B)])
